# revision 2
# baseline (speedup 1.0000x reference)
"""Channel-attention (CAM) kernel for Trainium2, 8 NeuronCores.

Reference computation (per batch b):
    A   = x[b].reshape(L, C)            # L = 48^3 = 110592, C = 256
    G   = A^T A                          # [C, C] Gram matrix
    S   = softmax(G, axis=-1)
    out = gamma * (A @ S) + x[b]

Sharding: L-parallel across the 8 cores (each core owns L/8 rows of both
batches).  Each core computes a partial Gram over its shard, an AllReduce
over all 8 cores completes the [2, C, C] Gram, every core redundantly
computes softmax (tiny), scales by gamma (so a zero gamma makes the second
matmul exactly zero), then computes its shard of A @ (gamma*S) + x.

The PE contracts along partitions, so the second matmul needs A^T tiles;
they are produced on the fly with identity-matmul transposes of the
(re-loaded) x tiles.  x is re-read in fp32 for the final residual add so
the output is bit-exact in x when gamma == 0.
"""

import numpy as np
from contextlib import ExitStack

import concourse.bass as bass
import concourse.tile as tile
from concourse import bacc, mybir
from concourse.bass import ts
from concourse.bass_utils import run_bass_kernel_spmd
from concourse.masks import make_identity

F32 = mybir.dt.float32
BF16 = mybir.dt.bfloat16
AF = mybir.ActivationFunctionType

N_CORES = 8
B = 2
L = 48 * 48 * 48          # 110592
C = 256
L_SH = L // N_CORES       # 13824 rows per core per batch
ROWS = B * L_SH           # 27648 rows per core
P = 128
TPB = L_SH // P           # 108 tiles per batch
T_TOT = B * TPB           # 216 tiles per core
NPRE = 24                 # phase-2 software-pipeline prologue depth

_CACHE: dict = {}


def _build():
    nc = bacc.Bacc(
        "TRN2", target_bir_lowering=False, debug=False, num_devices=N_CORES
    )
    x_dram = nc.dram_tensor("x", [ROWS, C], F32, kind="ExternalInput")
    g_dram = nc.dram_tensor("gamma", [1, 1], F32, kind="ExternalInput")
    o_dram = nc.dram_tensor("out", [ROWS, C], F32, kind="ExternalOutput")
    cc_in = nc.dram_tensor("cc_in", [4 * P, C], F32, kind="Internal")
    cc_out = nc.dram_tensor("cc_out", [4 * P, C], F32, kind="Internal")
    X, GAM, OUT = x_dram.ap(), g_dram.ap(), o_dram.ap()
    CCI, CCO = cc_in.ap(), cc_out.ap()

    with tile.TileContext(nc) as tc, ExitStack() as octx:
        constp = octx.enter_context(tc.tile_pool(name="const", bufs=1))
        ident = constp.tile([P, P], BF16, name="ident", tag="ident")
        make_identity(nc, ident[:])
        gam_sb = constp.tile([1, 1], F32, name="gam_sb", tag="gam_sb")
        nc.sync.dma_start(gam_sb[:], GAM[:, :])
        gam_bc = constp.tile([P, 1], F32, name="gam_bc", tag="gam_bc")
        nc.gpsimd.partition_broadcast(gam_bc[:], gam_sb[:])
        s_bf = [
            constp.tile([P, C], BF16, name=f"sbf{i}", tag=f"sbf{i}")
            for i in range(4)
        ]

        # ---- phase 1: partial Gram G = A^T A over the local L-shard ----
        with ExitStack() as p1:
            xp = p1.enter_context(tc.tile_pool(name="p1x", bufs=12))
            bp = p1.enter_context(tc.tile_pool(name="p1b", bufs=6))
            gp = p1.enter_context(tc.tile_pool(name="p1g", bufs=4))
            psg = p1.enter_context(
                tc.tile_pool(name="psg", bufs=1, space="PSUM")
            )
            g_ps = [
                psg.tile([P, C], F32, name=f"gps{i}", tag=f"gps{i}")
                for i in range(4)
            ]
            for t in range(T_TOT):
                b = t // TPB
                first = (t % TPB) == 0
                last = (t % TPB) == TPB - 1
                xt = xp.tile([P, C], F32, name="x1", tag="x1")
                nc.sync.dma_start(xt[:], X[ts(t, P), :])
                xb = bp.tile([P, C], BF16, name="xb1", tag="xb1")
                nc.vector.tensor_copy(xb[:], xt[:])
                nc.tensor.matmul(
                    g_ps[2 * b][:], xb[:, 0:P], xb[:], start=first, stop=last
                )
                nc.tensor.matmul(
                    g_ps[2 * b + 1][:], xb[:, P:C], xb[:], start=first, stop=last
                )
            for i in range(4):
                gsb = gp.tile([P, C], F32, name="gsb", tag="gsb")
                nc.scalar.activation(gsb[:], g_ps[i][:], AF.Copy)
                nc.sync.dma_start(CCI[ts(i, P), :], gsb[:])

        # ---- all-reduce the partial Grams across all 8 cores ----
        nc.gpsimd.collective_compute(
            "AllReduce",
            mybir.AluOpType.add,
            replica_groups=[list(range(N_CORES))],
            ins=[CCI[:, :]],
            outs=[CCO[:, :]],
        )

        # ---- softmax rows + fold gamma:  s_bf = gamma * softmax(G) ----
        with ExitStack() as sm:
            sp = sm.enter_context(tc.tile_pool(name="smx", bufs=4))
            for i in range(4):
                gf = sp.tile([P, C], F32, name="gf", tag="gf")
                nc.sync.dma_start(gf[:], CCO[ts(i, P), :])
                nmx = sp.tile([P, 1], F32, name="nmx", tag="nmx")
                nc.vector.tensor_reduce(
                    nmx[:],
                    gf[:],
                    axis=mybir.AxisListType.X,
                    op=mybir.AluOpType.max,
                    negate=True,
                )
                ex = sp.tile([P, C], F32, name="ex", tag="ex")
                ssum = sp.tile([P, 1], F32, name="ssum", tag="ssum")
                nc.scalar.activation(
                    ex[:], gf[:], AF.Exp, bias=nmx[:], scale=1.0, accum_out=ssum[:]
                )
                inv = sp.tile([P, 1], F32, name="inv", tag="inv")
                nc.vector.reciprocal(inv[:], ssum[:])
                sc = sp.tile([P, 1], F32, name="sc", tag="sc")
                nc.vector.tensor_mul(sc[:], inv[:], gam_bc[:])
                nc.scalar.activation(s_bf[i][:], ex[:], AF.Copy, scale=sc[:])

        # ---- phase 2: out = A @ s_bf + x  (software-pipelined) ----
        with ExitStack() as p2:
            xp2 = p2.enter_context(tc.tile_pool(name="p2x", bufs=NPRE + 4))
            bp2 = p2.enter_context(tc.tile_pool(name="p2b", bufs=6))
            ap2 = p2.enter_context(
                tc.tile_pool(name="p2a", bufs=2 * (NPRE + 4))
            )
            op2 = p2.enter_context(tc.tile_pool(name="p2o", bufs=12))
            pst = p2.enter_context(
                tc.tile_pool(name="pst", bufs=4, space="PSUM")
            )
            psy = p2.enter_context(
                tc.tile_pool(name="psy", bufs=3, space="PSUM")
            )

            xts, ats = {}, {}

            def prework(t):
                xt = xp2.tile([P, C], F32, name="x2", tag="x2")
                nc.sync.dma_start(xt[:], X[ts(t, P), :])
                xb = bp2.tile([P, C], BF16, name="xb2", tag="xb2")
                nc.vector.tensor_copy(xb[:], xt[:])
                tp0 = pst.tile([P, P], F32, name="tp0", tag="tp")
                tp1 = pst.tile([P, P], F32, name="tp1", tag="tp")
                nc.tensor.matmul(tp0[:], xb[:, 0:P], ident[:], start=True, stop=True)
                nc.tensor.matmul(tp1[:], xb[:, P:C], ident[:], start=True, stop=True)
                a0 = ap2.tile([P, P], BF16, name="at0", tag="at")
                a1 = ap2.tile([P, P], BF16, name="at1", tag="at")
                nc.scalar.activation(a0[:], tp0[:], AF.Copy)
                nc.scalar.activation(a1[:], tp1[:], AF.Copy)
                xts[t] = xt
                ats[t] = (a0, a1)

            def mainwork(t):
                b = t // TPB
                a0, a1 = ats.pop(t)
                xt = xts.pop(t)
                y = psy.tile([P, C], F32, name="y", tag="y")
                nc.tensor.matmul(
                    y[:], a0[:], s_bf[2 * b][:], start=True, stop=False
                )
                nc.tensor.matmul(
                    y[:], a1[:], s_bf[2 * b + 1][:], start=False, stop=True
                )
                ot = op2.tile([P, C], F32, name="ot", tag="ot")
                nc.vector.tensor_add(ot[:], y[:], xt[:])
                nc.sync.dma_start(OUT[ts(t, P), :], ot[:])

            for t in range(min(NPRE, T_TOT)):
                prework(t)
            for t in range(T_TOT):
                mainwork(t)
                if t + NPRE < T_TOT:
                    prework(t + NPRE)

    nc.compile()
    return nc


def _get_nc():
    if "nc" not in _CACHE:
        _CACHE["nc"] = _build()
    return _CACHE["nc"]


def kernel(x: np.ndarray, gamma: np.ndarray, **_kw) -> np.ndarray:
    nc = _get_nc()
    x = np.asarray(x, dtype=np.float32)
    orig_shape = x.shape
    x3 = x.reshape(B, L, C)
    gam = np.asarray(gamma, dtype=np.float32).reshape(1, 1)
    in_maps = []
    for k in range(N_CORES):
        shard = np.ascontiguousarray(
            x3[:, k * L_SH : (k + 1) * L_SH, :]
        ).reshape(ROWS, C)
        in_maps.append({"x": shard, "gamma": gam})
    res = run_bass_kernel_spmd(nc, in_maps, core_ids=list(range(N_CORES)))
    out = np.empty((B, L, C), dtype=np.float32)
    for k in range(N_CORES):
        out[:, k * L_SH : (k + 1) * L_SH, :] = res.results[k]["out"].reshape(
            B, L_SH, C
        )
    return out.reshape(orig_shape)


# revision 3
# speedup vs baseline: 1.6255x; 1.6255x over previous
"""Channel-attention (CAM) kernel for Trainium2, 8 NeuronCores.

Reference computation (per batch b):
    A   = x[b].reshape(L, C)            # L = 48^3 = 110592, C = 256
    G   = A^T A                          # [C, C] Gram matrix
    S   = softmax(G, axis=-1)
    out = gamma * (A @ S) + x[b]

Sharding: L-parallel across the 8 cores (each core owns L/8 rows of both
batches).  Each core computes a partial Gram over its shard, an AllReduce
(one per batch, so the first overlaps remaining phase-1 compute) completes
the [C, C] Grams, every core redundantly computes softmax (tiny), scales
by gamma (a zero gamma makes the second matmul exactly zero), then
computes its shard of A @ (gamma*S) + x.

Layout trick: tiles are loaded as [128, RPP, C] "supertiles" where
partition p holds RPP *consecutive* DRAM rows — DMA packets are RPP KB
instead of 1 KB.  The Gram contraction is invariant to the row
permutation, and the transpose / second matmul / residual add / store all
use the same permuted order consistently, so results land in the right
DRAM rows.

The PE contracts along partitions, so the second matmul needs A^T tiles;
they are produced on the fly with identity-matmul transposes of the
(re-loaded) x tiles.  x is re-read in fp32 for the final residual add so
the output is bit-exact in x when gamma == 0.
"""

import numpy as np
from contextlib import ExitStack

import concourse.bass as bass
import concourse.tile as tile
from concourse import bacc, mybir
from concourse.bass import ts
from concourse.bass_utils import run_bass_kernel_spmd
from concourse.masks import make_identity

F32 = mybir.dt.float32
BF16 = mybir.dt.bfloat16
AF = mybir.ActivationFunctionType

N_CORES = 8
B = 2
L = 48 * 48 * 48          # 110592
C = 256
L_SH = L // N_CORES       # 13824 rows per core per batch
ROWS = B * L_SH           # 27648 rows per core
P = 128
RPP = 4                   # rows per partition per supertile
SROWS = P * RPP           # 512 rows per supertile
SPB = L_SH // SROWS       # 27 supertiles per batch
S_TOT = B * SPB           # 54 supertiles per core
NPRE = 12                 # phase-2 software-pipeline prologue (supertiles)

_CACHE: dict = {}


def _build():
    nc = bacc.Bacc(
        "TRN2", target_bir_lowering=False, debug=False, num_devices=N_CORES
    )
    x_dram = nc.dram_tensor("x", [ROWS, C], F32, kind="ExternalInput")
    g_dram = nc.dram_tensor("gamma", [1, 1], F32, kind="ExternalInput")
    o_dram = nc.dram_tensor("out", [ROWS, C], F32, kind="ExternalOutput")
    cc_in = [
        nc.dram_tensor(f"cc_in{b}", [2 * P, C], F32, kind="Internal")
        for b in range(B)
    ]
    cc_out = [
        nc.dram_tensor(f"cc_out{b}", [2 * P, C], F32, kind="Internal")
        for b in range(B)
    ]
    X, GAM, OUT = x_dram.ap(), g_dram.ap(), o_dram.ap()

    def x_super(s):
        return X[ts(s, SROWS), :].rearrange("(p j) c -> p j c", j=RPP)

    def o_super(s):
        return OUT[ts(s, SROWS), :].rearrange("(p j) c -> p j c", j=RPP)

    with tile.TileContext(nc) as tc, ExitStack() as octx:
        constp = octx.enter_context(tc.tile_pool(name="const", bufs=1))
        ident = constp.tile([P, P], BF16, name="ident", tag="ident")
        make_identity(nc, ident[:])
        gam_sb = constp.tile([1, 1], F32, name="gam_sb", tag="gam_sb")
        nc.sync.dma_start(gam_sb[:], GAM[:, :])
        gam_bc = constp.tile([P, 1], F32, name="gam_bc", tag="gam_bc")
        nc.gpsimd.partition_broadcast(gam_bc[:], gam_sb[:])
        s_bf = [
            constp.tile([P, C], BF16, name=f"sbf{i}", tag=f"sbf{i}")
            for i in range(4)
        ]

        # ---- phase 1: partial Gram G = A^T A over the local L-shard ----
        p1 = octx.enter_context(ExitStack())
        xp = p1.enter_context(tc.tile_pool(name="p1x", bufs=6))
        bp = p1.enter_context(tc.tile_pool(name="p1b", bufs=4))
        gp = p1.enter_context(tc.tile_pool(name="p1g", bufs=4))
        psg = p1.enter_context(tc.tile_pool(name="psg", bufs=1, space="PSUM"))
        g_ps = [
            psg.tile([P, C], F32, name=f"gps{i}", tag=f"gps{i}")
            for i in range(4)
        ]

        def phase1_batch(b):
            for si in range(SPB):
                s = b * SPB + si
                xt = xp.tile([P, RPP, C], F32, name="x1", tag="x1")
                nc.sync.dma_start(xt[:], x_super(s))
                xb = bp.tile([P, RPP, C], BF16, name="xb1", tag="xb1")
                nc.vector.tensor_copy(xb[:], xt[:])
                for j in range(RPP):
                    first = si == 0 and j == 0
                    last = si == SPB - 1 and j == RPP - 1
                    nc.tensor.matmul(
                        g_ps[2 * b][:], xb[:, j, 0:P], xb[:, j, :],
                        start=first, stop=last,
                    )
                    nc.tensor.matmul(
                        g_ps[2 * b + 1][:], xb[:, j, P:C], xb[:, j, :],
                        start=first, stop=last,
                    )
            # stage partial Gram for this batch and kick its AllReduce
            for m in range(2):
                gsb = gp.tile([P, C], F32, name="gsb", tag="gsb")
                nc.scalar.activation(gsb[:], g_ps[2 * b + m][:], AF.Copy)
                nc.sync.dma_start(cc_in[b].ap()[ts(m, P), :], gsb[:])
            nc.gpsimd.collective_compute(
                "AllReduce",
                mybir.AluOpType.add,
                replica_groups=[list(range(N_CORES))],
                ins=[cc_in[b].ap()[:, :]],
                outs=[cc_out[b].ap()[:, :]],
            )

        phase1_batch(0)
        phase1_batch(1)
        p1.close()

        # ---- softmax rows + fold gamma:  s_bf = gamma * softmax(G) ----
        with ExitStack() as sm:
            sp = sm.enter_context(tc.tile_pool(name="smx", bufs=4))
            for i in range(4):
                b, m = i // 2, i % 2
                gf = sp.tile([P, C], F32, name="gf", tag="gf")
                nc.sync.dma_start(gf[:], cc_out[b].ap()[ts(m, P), :])
                nmx = sp.tile([P, 1], F32, name="nmx", tag="nmx")
                nc.vector.tensor_reduce(
                    nmx[:],
                    gf[:],
                    axis=mybir.AxisListType.X,
                    op=mybir.AluOpType.max,
                    negate=True,
                )
                ex = sp.tile([P, C], F32, name="ex", tag="ex")
                ssum = sp.tile([P, 1], F32, name="ssum", tag="ssum")
                nc.scalar.activation(
                    ex[:], gf[:], AF.Exp, bias=nmx[:], scale=1.0, accum_out=ssum[:]
                )
                inv = sp.tile([P, 1], F32, name="inv", tag="inv")
                nc.vector.reciprocal(inv[:], ssum[:])
                sc = sp.tile([P, 1], F32, name="sc", tag="sc")
                nc.vector.tensor_mul(sc[:], inv[:], gam_bc[:])
                nc.scalar.activation(s_bf[i][:], ex[:], AF.Copy, scale=sc[:])

        # ---- phase 2: out = A @ s_bf + x  (software-pipelined) ----
        with ExitStack() as p2:
            xp2 = p2.enter_context(tc.tile_pool(name="p2x", bufs=NPRE + 3))
            bp2 = p2.enter_context(tc.tile_pool(name="p2b", bufs=4))
            ap2 = p2.enter_context(
                tc.tile_pool(name="p2a", bufs=2 * RPP * (NPRE + 2))
            )
            op2 = p2.enter_context(tc.tile_pool(name="p2o", bufs=4))
            pst = p2.enter_context(tc.tile_pool(name="pst", bufs=4, space="PSUM"))
            psy = p2.enter_context(tc.tile_pool(name="psy", bufs=2, space="PSUM"))

            xts, ats = {}, {}

            def prework(s):
                xt = xp2.tile([P, RPP, C], F32, name="x2", tag="x2")
                nc.sync.dma_start(xt[:], x_super(s))
                xb = bp2.tile([P, RPP, C], BF16, name="xb2", tag="xb2")
                nc.vector.tensor_copy(xb[:], xt[:])
                aa = []
                for j in range(RPP):
                    tp0 = pst.tile([P, P], F32, name="tp0", tag="tp")
                    tp1 = pst.tile([P, P], F32, name="tp1", tag="tp")
                    nc.tensor.matmul(
                        tp0[:], xb[:, j, 0:P], ident[:], start=True, stop=True
                    )
                    nc.tensor.matmul(
                        tp1[:], xb[:, j, P:C], ident[:], start=True, stop=True
                    )
                    a0 = ap2.tile([P, P], BF16, name="at0", tag="at")
                    a1 = ap2.tile([P, P], BF16, name="at1", tag="at")
                    nc.scalar.activation(a0[:], tp0[:], AF.Copy)
                    nc.scalar.activation(a1[:], tp1[:], AF.Copy)
                    aa.append((a0, a1))
                xts[s] = xt
                ats[s] = aa

            def mainwork(s):
                b = s // SPB
                aa = ats.pop(s)
                xt = xts.pop(s)
                y = psy.tile([P, RPP, C], F32, name="y", tag="y")
                for j in range(RPP):
                    a0, a1 = aa[j]
                    nc.tensor.matmul(
                        y[:, j, :], a0[:], s_bf[2 * b][:], start=True, stop=False
                    )
                    nc.tensor.matmul(
                        y[:, j, :], a1[:], s_bf[2 * b + 1][:],
                        start=False, stop=True,
                    )
                ot = op2.tile([P, RPP, C], F32, name="ot", tag="ot")
                nc.vector.tensor_add(ot[:], y[:], xt[:])
                nc.gpsimd.dma_start(o_super(s), ot[:])

            for s in range(min(NPRE, S_TOT)):
                prework(s)
            for s in range(S_TOT):
                mainwork(s)
                if s + NPRE < S_TOT:
                    prework(s + NPRE)

    nc.compile()
    return nc


def _get_nc():
    if "nc" not in _CACHE:
        _CACHE["nc"] = _build()
    return _CACHE["nc"]


def kernel(x: np.ndarray, gamma: np.ndarray, **_kw) -> np.ndarray:
    nc = _get_nc()
    x = np.asarray(x, dtype=np.float32)
    orig_shape = x.shape
    x3 = x.reshape(B, L, C)
    gam = np.asarray(gamma, dtype=np.float32).reshape(1, 1)
    in_maps = []
    for k in range(N_CORES):
        shard = np.ascontiguousarray(
            x3[:, k * L_SH : (k + 1) * L_SH, :]
        ).reshape(ROWS, C)
        in_maps.append({"x": shard, "gamma": gam})
    res = run_bass_kernel_spmd(nc, in_maps, core_ids=list(range(N_CORES)))
    out = np.empty((B, L, C), dtype=np.float32)
    for k in range(N_CORES):
        out[:, k * L_SH : (k + 1) * L_SH, :] = res.results[k]["out"].reshape(
            B, L_SH, C
        )
    return out.reshape(orig_shape)
